# revision 8
# baseline (speedup 1.0000x reference)
"""GCN (3x GCNConv + mean-pool + LayerNorm + dense + Bayesian head) on 8
Trainium2 NeuronCores.

Strategy: nodes sharded by dst across 8 cores (12500 nodes/core grouped into
98 blocks of 128). Each conv layer = indirect-DMA gather of source-node rows
from a replicated table + PSUM-accumulated matmuls against per-chunk one-hot
selection matrices (norm folded in, built on-device on the VectorEngine from
per-edge scalars). Weights are applied after aggregation (A@(XW) = (A@X)@W),
and the next layer's gather table is premultiplied (H@W_next) before an
AllGather. Pooling = matmul with a one-hot (graph, 1/cnt) matrix, AllReduce,
then the tiny LayerNorm/dense/Bayesian head replicated on every core.
"""
import sys
sys.path.insert(0, '/opt/trn_rl_repo')
import numpy as np

from concourse import bass, bacc, tile, mybir
from concourse import bass_utils
from concourse.bass import IndirectOffsetOnAxis

# problem dims (hardcoded per spec)
N, E, F0, F1, F2, F3, B = 100000, 640000, 128, 128, 64, 32, 64
C = 8            # cores
P = 128          # partitions
NPC = N // C     # 12500 nodes per core
NB = (NPC + P - 1) // P   # 98 blocks per core
SLOTS = NB * P   # 12544 table rows per core
NTBL = C * SLOTS

F32 = mybir.dt.float32
I32 = mybir.dt.int32


# ---------------------------------------------------------------- host prep
def preprocess(x, edge_index, batch, edge_weight):
    src = np.asarray(edge_index[0], dtype=np.int64)
    dst = np.asarray(edge_index[1], dtype=np.int64)
    ew = np.asarray(edge_weight, dtype=np.float64)
    batch = np.asarray(batch, dtype=np.int64)

    loop = np.arange(N, dtype=np.int64)
    src = np.concatenate([src, loop])
    dst = np.concatenate([dst, loop])
    ew = np.concatenate([ew, np.ones(N)])

    deg = np.bincount(dst, weights=ew, minlength=N)
    dinv = np.where(deg > 0, 1.0 / np.sqrt(np.maximum(deg, 1e-12)), 0.0)
    norm = (dinv[src] * ew * dinv[dst]).astype(np.float32)

    dst_core = dst // NPC
    dst_q = dst % NPC
    dst_b = dst_q // P
    dst_lane = dst_q % P

    # per-core block edge counts -> block permutation (sorted desc) so the
    # chunk schedule is uniform across cores
    cnts = np.zeros((C, NB), dtype=np.int64)
    np.add.at(cnts, (dst_core, dst_b), 1)
    order = np.argsort(-cnts, axis=1, kind='stable')          # pos -> block
    posof = np.empty_like(order)                              # block -> pos
    for c in range(C):
        posof[c, order[c]] = np.arange(NB)
    sorted_cnts = np.take_along_axis(cnts, order, axis=1)     # [C, NB] desc
    need = (sorted_cnts + P - 1) // P
    CH = np.maximum(need.max(axis=0), 1).astype(np.int64)     # [NB]
    cumCH = np.concatenate([[0], np.cumsum(CH)])
    TOTCH = int(cumCH[-1])

    # table row of every global node
    n_all = np.arange(N, dtype=np.int64)
    core_of = n_all // NPC
    q = n_all % NPC
    tblrow = core_of * SLOTS + posof[core_of, q // P] * P + (q % P)

    # assign edges to (core, pos, chunk, lane)
    epos = posof[dst_core, dst_b]
    sort_idx = np.lexsort((epos, dst_core))
    s_core = dst_core[sort_idx]
    s_pos = epos[sort_idx]
    # rank within each (core, pos) group
    grp = s_core * NB + s_pos
    grp_start = np.zeros(C * NB, dtype=np.int64)
    np.add.at(grp_start, grp, 1)
    starts = np.concatenate([[0], np.cumsum(grp_start)])[:-1]
    j = np.arange(len(sort_idx)) - starts[grp]
    lane = j % P
    col = cumCH[s_pos] + j // P

    gidx = np.zeros((C, P, TOTCH), dtype=np.int32)
    dl = np.zeros((C, P, TOTCH), dtype=np.float32)
    nrm = np.zeros((C, P, TOTCH), dtype=np.float32)
    e = sort_idx
    gidx[s_core, lane, col] = tblrow[src[e]].astype(np.int32)
    dl[s_core, lane, col] = dst_lane[e].astype(np.float32)
    nrm[s_core, lane, col] = norm[e]

    # pooling: per slot -> graph lane + 1/cnt
    cnt = np.bincount(batch, minlength=B).astype(np.float64)
    icnt = (1.0 / np.maximum(cnt, 1.0)).astype(np.float32)
    gl = np.zeros((C, P, NB), dtype=np.float32)
    ic = np.zeros((C, P, NB), dtype=np.float32)
    for c in range(C):
        nodes = c * NPC + order[c][:, None] * P + np.arange(P)[None, :]  # [NB, P]
        valid = (order[c][:, None] * P + np.arange(P)[None, :]) < NPC
        nn = np.where(valid, nodes, 0)
        g = batch[nn]
        gl[c] = np.where(valid, g, 0).T.astype(np.float32)
        ic[c] = np.where(valid, icnt[g], 0.0).T.astype(np.float32)

    # x staged in table order
    xtab = np.zeros((NTBL, F0), dtype=np.float32)
    xtab[tblrow] = np.asarray(x, dtype=np.float32)

    return dict(gidx=gidx, dl=dl, nrm=nrm, gl=gl, ic=ic, xtab=xtab,
                CH=CH, cumCH=cumCH, TOTCH=TOTCH)


# ------------------------------------------------------------- bass builder
def build_bass(pp_data, weights, dt_tab=mybir.dt.bfloat16, reps=1):
    """weights: dict of numpy arrays (W1,b1,...). Returns (nc, in_map_common).

    in_map_common holds inputs identical on all cores; per-core inputs
    (gidx/dl/nrm/gl/ic) are added by the caller.
    """
    CH = pp_data['CH']; cumCH = pp_data['cumCH']; TOTCH = pp_data['TOTCH']
    np_tab = mybir.dt.np(dt_tab)

    nc = bacc.Bacc("TRN2", target_bir_lowering=False, debug=False,
                   enable_asserts=False, num_devices=C)

    def ein(name, shape, dt):
        return nc.dram_tensor(name, shape, dt, kind="ExternalInput").ap()

    xtab_d = ein("xtab", [NTBL, F0], dt_tab)
    gidx_d = ein("gidx", [P, TOTCH], I32)
    dl_d = ein("dl", [P, TOTCH], F32)
    nrm_d = ein("nrm", [P, TOTCH], F32)
    gl_d = ein("gl", [P, NB], F32)
    ic_d = ein("ic", [P, NB], F32)
    iota_d = ein("iota", [P, P], F32)
    ident_d = ein("ident", [P, P], F32)
    ones_d = ein("ones", [1, P], F32)
    W1_d = ein("W1", [F0, F1], F32)
    W2_d = ein("W2", [F1, F2], F32)
    W3_d = ein("W3", [F2, F3], F32)
    b1_d = ein("b1", [P, F1], F32)
    b2_d = ein("b2", [P, F2], F32)
    b3_d = ein("b3", [P, F3], F32)
    lng_d = ein("lng", [B, F3], F32)
    lnb_d = ein("lnb", [B, F3], F32)
    fcw_d = ein("fcw", [F3, 8], F32)
    fcb_d = ein("fcb", [1, 8], F32)
    wmu_d = ein("wmu", [1, 8], F32)
    wrho_d = ein("wrho", [1, 8], F32)
    weps_d = ein("weps", [1, 8], F32)
    bmu_d = ein("bmu", [1, 1], F32)
    brho_d = ein("brho", [1, 1], F32)
    beps_d = ein("beps", [1, 1], F32)

    out_d = nc.dram_tensor("out", [B, 1], F32, kind="ExternalOutput").ap()

    AF = mybir.ActivationFunctionType
    OP = mybir.AluOpType
    RG = [list(range(C))]

    with tile.TileContext(nc) as tc:
        with tc.tile_pool(name="const", bufs=1) as cp, \
             tc.tile_pool(name="sb", bufs=3) as sb, \
             tc.tile_pool(name="gpool", bufs=8) as gp, \
             tc.tile_pool(name="spool", bufs=8) as spl, \
             tc.tile_pool(name="ps_agg", bufs=2, space="PSUM") as ps_agg, \
             tc.tile_pool(name="ps_out", bufs=2, space="PSUM") as ps_out, \
             tc.tile_pool(name="ps_misc", bufs=1, space="PSUM") as ps_misc, \
             tc.tile_pool(name="ps_pool", bufs=1, space="PSUM") as ps_pool, \
             tc.tile_pool(name="dram", bufs=1, space="DRAM") as dp:

            def load_const(ap_d, shape, dt=F32, name=None):
                t = cp.tile(shape, dt, name=name or ap_d.tensor.name + "_sb")
                nc.sync.dma_start(t[:], ap_d)
                return t

            gidx_sb = load_const(gidx_d, [P, TOTCH], I32)
            dl_sb = load_const(dl_d, [P, TOTCH])
            nrm_sb = load_const(nrm_d, [P, TOTCH])
            gl_sb = load_const(gl_d, [P, NB])
            ic_sb = load_const(ic_d, [P, NB])
            iota_sb = load_const(iota_d, [P, P])
            ident_sb = load_const(ident_d, [P, P])
            ones_sb = load_const(ones_d, [1, P])
            W1_sb = load_const(W1_d, [F0, F1])
            W2_sb = load_const(W2_d, [F1, F2])
            W3_sb = load_const(W3_d, [F2, F3])
            b_sb = [load_const(b1_d, [P, F1]), load_const(b2_d, [P, F2]),
                    load_const(b3_d, [P, F3])]
            lng_sb = load_const(lng_d, [B, F3])
            lnb_sb = load_const(lnb_d, [B, F3])
            fcw_sb = load_const(fcw_d, [F3, 8])
            fcb_sb = load_const(fcb_d, [1, 8])
            wmu_sb = load_const(wmu_d, [1, 8])
            wrho_sb = load_const(wrho_d, [1, 8])
            weps_sb = load_const(weps_d, [1, 8])
            bmu_sb = load_const(bmu_d, [1, 1])
            brho_sb = load_const(brho_d, [1, 1])
            beps_sb = load_const(beps_d, [1, 1])

            def elu_into(h, x_ps, F):
                """h (SBUF) = elu(x_ps + bias already applied)."""
                t1 = sb.tile([P, F], F32, tag="elu1", name="t1")
                nc.vector.tensor_scalar_min(t1[:], x_ps, 0.0)
                t2 = sb.tile([P, F], F32, tag="elu2", name="t2")
                nc.scalar.activation(t2[:], t1[:], AF.Exp)
                nc.vector.tensor_scalar_add(t2[:], t2[:], -1.0)
                nc.vector.tensor_tensor(h, x_ps, t2[:], op=OP.max)

            for rep in range(reps):
                tabs = [xtab_d]
                cc_in = []
                for (fdim, nm) in ((F2, "t2"), (F3, "t3")):
                    ci = dp.tile([SLOTS, fdim], dt_tab, name=f"ccin_{nm}_{rep}")
                    to = dp.tile([NTBL, fdim], dt_tab, name=f"tab_{nm}_{rep}")
                    cc_in.append(ci)
                    tabs.append(to)

                pool_ps = ps_pool.tile([B, F3], F32, tag="pp", name="pool_ps")

                for L in range(3):
                    Fin = (F0, F2, F3)[L]
                    Fout = (F1, F2, F3)[L]
                    tab = tabs[L]
                    for pos in range(NB):
                        Chp = int(CH[pos]); c0 = int(cumCH[pos])
                        aggT = ps_agg.tile([Fin, P], F32, tag="aggT",
                                           name="aggT")
                        for k in range(Chp):
                            G = gp.tile([P, Fin], dt_tab, tag="G", name="G")
                            nc.gpsimd.indirect_dma_start(
                                out=G[:], out_offset=None,
                                in_=tab[:, :] if L > 0 else tab,
                                in_offset=IndirectOffsetOnAxis(
                                    ap=gidx_sb[:, c0 + k:c0 + k + 1], axis=0),
                            )
                            S_b = spl.tile([P, P], dt_tab, tag="S", name="S_b")
                            nc.vector.tensor_scalar(
                                out=S_b[:], in0=iota_sb[:],
                                scalar1=dl_sb[:, c0 + k:c0 + k + 1],
                                scalar2=nrm_sb[:, c0 + k:c0 + k + 1],
                                op0=OP.is_equal, op1=OP.mult)
                            nc.tensor.matmul(
                                out=aggT[:], lhsT=G[:],
                                rhs=S_b[:], start=(k == 0), stop=(k == Chp - 1))
                        aggT_sb = sb.tile([Fin, P], F32, tag="aggTs",
                                          name="aggT_sb")
                        nc.vector.tensor_copy(aggT_sb[:], aggT[:])
                        out_ps = ps_out.tile([P, Fout], F32, tag="ops",
                                             name="out_ps")
                        if L == 0:
                            nc.tensor.matmul(out=out_ps[:], lhsT=aggT_sb[:],
                                             rhs=W1_sb[:], start=True,
                                             stop=True)
                        else:
                            nc.tensor.transpose(out=out_ps[:], in_=aggT_sb[:],
                                                identity=ident_sb[:Fin, :Fin])
                        xb = sb.tile([P, Fout], F32, tag="xb", name="xb")
                        nc.vector.tensor_tensor(xb[:], out_ps[:],
                                                b_sb[L][:], op=OP.add)
                        h = sb.tile([P, Fout], F32, tag="h", name="h")
                        elu_into(h[:], xb[:], Fout)
                        if L < 2:
                            Fnext = (F2, F3)[L]
                            Wn = (W2_sb, W3_sb)[L]
                            trp = ps_misc.tile([Fout, P], F32, tag="trp",
                                               name="trp")
                            nc.tensor.transpose(out=trp[:], in_=h[:],
                                                identity=ident_sb[:])
                            trs = sb.tile([Fout, P], F32, tag="trs",
                                          name="trs")
                            nc.vector.tensor_copy(trs[:], trp[:])
                            pm = ps_misc.tile([P, Fnext], F32, tag="pm",
                                              name="pm")
                            nc.tensor.matmul(out=pm[:], lhsT=trs[:],
                                             rhs=Wn[:], start=True, stop=True)
                            pms = sb.tile([P, Fnext], dt_tab, tag="pms",
                                          name="pms")
                            nc.vector.tensor_copy(pms[:], pm[:])
                            nc.sync.dma_start(
                                cc_in[L][pos * P:(pos + 1) * P, :], pms[:])
                        else:
                            Sp = spl.tile([P, B], F32, tag="Sp", name="Sp")
                            nc.vector.tensor_scalar(
                                out=Sp[:], in0=iota_sb[:, :B],
                                scalar1=gl_sb[:, pos:pos + 1],
                                scalar2=ic_sb[:, pos:pos + 1],
                                op0=OP.is_equal, op1=OP.mult)
                            nc.tensor.matmul(out=pool_ps[:], lhsT=Sp[:],
                                             rhs=h[:], start=(pos == 0),
                                             stop=(pos == NB - 1),
                                             skip_group_check=True)
                    if L < 2:
                        nc.gpsimd.collective_compute(
                            "AllGather", OP.bypass, replica_groups=RG,
                            ins=[cc_in[L].opt()], outs=[tabs[L + 1].opt()])

                # ---- pooled mean allreduce + head
                pool_sb = sb.tile([B, F3], F32, tag="pool_sb", name="pool_sb")
                nc.vector.tensor_copy(pool_sb[:], pool_ps[:])
                ar_in = dp.tile([B, F3], F32, name=f"arin_{rep}")
                ar_out = dp.tile([B, F3], F32, name=f"arout_{rep}")
                nc.sync.dma_start(ar_in[:], pool_sb[:])
                nc.gpsimd.collective_compute(
                    "AllReduce", OP.add, replica_groups=RG,
                    ins=[ar_in.opt()], outs=[ar_out.opt()])
                gm = sb.tile([B, F3], F32, tag="gm", name="gm")
                nc.sync.dma_start(gm[:], ar_out[:])

                # LayerNorm over 32 features
                mu = sb.tile([B, 1], F32, tag="mu", name="mu")
                nc.vector.reduce_sum(out=mu[:], in_=gm[:],
                                     axis=mybir.AxisListType.X)
                nc.vector.tensor_scalar_mul(mu[:], mu[:], 1.0 / F3)
                xc = sb.tile([B, F3], F32, tag="xc", name="xc")
                nc.vector.tensor_scalar(out=xc[:], in0=gm[:], scalar1=mu[:],
                                        scalar2=None, op0=OP.subtract)
                sq = sb.tile([B, F3], F32, tag="sq", name="sq")
                nc.scalar.activation(sq[:], xc[:], AF.Square)
                vv = sb.tile([B, 1], F32, tag="vv", name="vv")
                nc.vector.reduce_sum(out=vv[:], in_=sq[:],
                                     axis=mybir.AxisListType.X)
                nc.vector.tensor_scalar_mul(vv[:], vv[:], 1.0 / F3)
                nc.vector.tensor_scalar_add(vv[:], vv[:], 1e-5)
                sd = sb.tile([B, 1], F32, tag="sd", name="sd")
                nc.scalar.activation(sd[:], vv[:], AF.Sqrt)
                rs = sb.tile([B, 1], F32, tag="rs", name="rs")
                nc.vector.reciprocal(rs[:], sd[:])
                nc.vector.tensor_scalar_mul(xc[:], xc[:], rs[:])
                y = sb.tile([B, F3], F32, tag="y", name="y")
                nc.vector.tensor_tensor(y[:], xc[:], lng_sb[:], op=OP.mult)
                nc.vector.tensor_tensor(y[:], y[:], lnb_sb[:], op=OP.add)

                # h2 = elu(y @ fc_w + fc_b)
                yT_ps = ps_misc.tile([F3, B], F32, tag="trp", name="yT_ps")
                nc.tensor.transpose(out=yT_ps[:], in_=y[:],
                                    identity=ident_sb[:B, :B])
                yT = sb.tile([F3, B], F32, tag="yTs", name="yT")
                nc.vector.tensor_copy(yT[:], yT_ps[:])
                h2_ps = ps_out.tile([B, 8], F32, tag="ops", name="h2_ps")
                nc.tensor.matmul(out=h2_ps[:], lhsT=yT[:], rhs=fcw_sb[:],
                                 start=True, stop=False)
                nc.tensor.matmul(out=h2_ps[:], lhsT=ones_sb[:1, :B],
                                 rhs=fcb_sb[:], start=False, stop=True)
                h2 = sb.tile([B, 8], F32, tag="h2s", name="h2")
                t1 = sb.tile([B, 8], F32, tag="ht1", name="ht1")
                nc.vector.tensor_scalar_min(t1[:], h2_ps[:], 0.0)
                t2 = sb.tile([B, 8], F32, tag="ht2", name="ht2")
                nc.scalar.activation(t2[:], t1[:], AF.Exp)
                nc.vector.tensor_scalar_add(t2[:], t2[:], -1.0)
                nc.vector.tensor_tensor(h2[:], h2_ps[:], t2[:], op=OP.max)

                # bayes weights: w = mu + softplus(rho) * eps
                # softplus(x) = ln(1 + exp(x)) (no Softplus ACT table here)
                weff = sb.tile([1, 8], F32, tag="weff", name="weff")
                nc.scalar.activation(weff[:], wrho_sb[:], AF.Exp)
                nc.vector.tensor_scalar_add(weff[:], weff[:], 1.0)
                nc.scalar.activation(weff[:], weff[:], AF.Ln)
                nc.vector.tensor_tensor(weff[:], weff[:], weps_sb[:],
                                        op=OP.mult)
                nc.vector.tensor_tensor(weff[:], weff[:], wmu_sb[:],
                                        op=OP.add)
                beff = sb.tile([1, 1], F32, tag="beff", name="beff")
                nc.scalar.activation(beff[:], brho_sb[:], AF.Exp)
                nc.vector.tensor_scalar_add(beff[:], beff[:], 1.0)
                nc.scalar.activation(beff[:], beff[:], AF.Ln)
                nc.vector.tensor_tensor(beff[:], beff[:], beps_sb[:],
                                        op=OP.mult)
                nc.vector.tensor_tensor(beff[:], beff[:], bmu_sb[:],
                                        op=OP.add)
                wb_ps = ps_misc.tile([B, 8], F32, tag="pm", name="wb_ps")
                nc.tensor.matmul(out=wb_ps[:], lhsT=ones_sb[:1, :B],
                                 rhs=weff[:], start=True, stop=True)
                bb_ps = ps_misc.tile([B, 1], F32, tag="pm", name="bb_ps")
                nc.tensor.matmul(out=bb_ps[:], lhsT=ones_sb[:1, :B],
                                 rhs=beff[:], start=True, stop=True)
                prod = sb.tile([B, 8], F32, tag="prod", name="prod")
                nc.vector.tensor_tensor(prod[:], h2[:], wb_ps[:], op=OP.mult)
                red = sb.tile([B, 1], F32, tag="red", name="red")
                nc.vector.reduce_sum(out=red[:], in_=prod[:],
                                     axis=mybir.AxisListType.X)
                res = sb.tile([B, 1], F32, tag="res", name="res")
                nc.vector.tensor_tensor(res[:], red[:], bb_ps[:], op=OP.add)
                nc.sync.dma_start(out_d, res[:])

    nc.compile()

    np_tab_t = mybir.dt.np(dt_tab)
    common = dict(
        xtab=pp_data['xtab'].astype(np_tab_t),
        iota=np.tile(np.arange(P, dtype=np.float32), (P, 1)),
        ident=np.eye(P, dtype=np.float32),
        ones=np.ones((1, P), dtype=np.float32),
        W1=weights['W1'].astype(np.float32),
        W2=weights['W2'].astype(np.float32),
        W3=weights['W3'].astype(np.float32),
        b1=np.tile(weights['b1'][None, :], (P, 1)).astype(np.float32),
        b2=np.tile(weights['b2'][None, :], (P, 1)).astype(np.float32),
        b3=np.tile(weights['b3'][None, :], (P, 1)).astype(np.float32),
        lng=np.tile(weights['ln_g'][None, :], (B, 1)).astype(np.float32),
        lnb=np.tile(weights['ln_b'][None, :], (B, 1)).astype(np.float32),
        fcw=weights['fc_w'].astype(np.float32),
        fcb=weights['fc_b'][None, :].astype(np.float32),
        wmu=weights['w_mu'].astype(np.float32),
        wrho=weights['w_rho'].astype(np.float32),
        weps=weights['w_eps'].astype(np.float32),
        bmu=weights['b_mu'][None, :].astype(np.float32),
        brho=weights['b_rho'][None, :].astype(np.float32),
        beps=weights['b_eps'][None, :].astype(np.float32),
    )
    return nc, common


def make_in_maps(pp_data, common):
    in_maps = []
    for c in range(C):
        m = dict(common)
        m['gidx'] = pp_data['gidx'][c]
        m['dl'] = pp_data['dl'][c]
        m['nrm'] = pp_data['nrm'][c]
        m['gl'] = pp_data['gl'][c]
        m['ic'] = pp_data['ic'][c]
        in_maps.append(m)
    return in_maps


DT_TAB = mybir.dt.bfloat16


def kernel(**inputs):
    x = np.asarray(inputs['x'])
    pp = preprocess(x, inputs['edge_index'], inputs['batch'],
                    inputs['edge_weight'])
    weights = {k: np.asarray(v) for k, v in inputs.items()
               if k not in ('x', 'edge_index', 'batch', 'edge_weight')}
    nc, common = build_bass(pp, weights, dt_tab=DT_TAB, reps=1)
    in_maps = make_in_maps(pp, common)
    res = bass_utils.run_bass_kernel_spmd(nc, in_maps, core_ids=list(range(C)))
    return res.results[0]['out'].astype(np.float32)
